# revision 4
# baseline (speedup 1.0000x reference)
"""Local (3x3 grid-neighborhood) attention for Trainium2, SPMD over 8 cores.

Problem: B=16, N=1024 (32x32 grid), C=1024, H=16 heads, D=64.
  qkv = x @ w_qkv.T; per-head local attention (each token attends to its
  3x3 grid neighborhood); out = attn_out @ w_proj.T + b_proj.

Sharding: data-parallel over batch, 2 batches per core, no collectives.

Device kernel structure (per batch):
  - q^T,k^T = (w_qkv.T).T-slices @ x^T   (fp32r matmuls, heads-transposed
    layout (d, n) so attention slices need no on-device transposes)
  - v = x @ w_v.T in natural (n, d) layout, then SBUF->SBUF DMA into
    overlapped 96-token "window half" tiles with a ones-column per head
    slot (for the softmax denominator via matmul).
  - attention per (head, 128-query tile): keys live in a 192-token window
    = 2 x 96-token halves.  S^T = K Q^T via PE (keys on partitions);
    P = exp(0.125*S^T) on ScalarE fused scale; 0/1 sparsity mask multiply
    on VectorE; O'^T = [V|1]^T P^T via PE accumulating both halves; row 64
    is the softmax sum; normalize rows 0..63 with reciprocal + partition
    broadcast.  All attention operands fp16 (values verified in range).
  - y = O^T-tiles.T @ w_proj.T (fp32r), DMA to DRAM.
"""

import numpy as np

B, N, C, H, D = 16, 1024, 1024, 16, 64
WS = 32
NCORES = 8
BPC = B // NCORES  # batches per core
KO = C // 128      # contraction tiles

_cache = {}


def _win_start(t):
    # key-window start (tokens) for query tile t (queries [128t, 128t+128))
    return max(0, min(32 * (4 * t - 1), N - 192))


def _half_starts():
    s = []
    for t in range(8):
        w = _win_start(t)
        for h in (w, w + 96):
            if h not in s:
                s.append(h)
    return sorted(s)


def _build_masks(np_dtype):
    # 0/1 masks in S^T layout: (3 variants, 96 keys, 2 halves x 128 queries)
    masks = np.zeros((3, 96, 256), dtype=np.float32)
    for vi, t in enumerate([0, 3, 7]):
        w = _win_start(t)
        q = 128 * t + np.arange(128)
        qr, qc = q // WS, q % WS
        for half in range(2):
            k = w + 96 * half + np.arange(96)
            kr, kc = k // WS, k % WS
            valid = (np.abs(kr[:, None] - qr[None, :]) <= 1) & (
                np.abs(kc[:, None] - qc[None, :]) <= 1)
            masks[vi, :, 128 * half:128 * half + 128] = valid
    return masks.astype(np_dtype)


def build_nc(bpc=BPC, heads=H, qtiles=8):
    import concourse.bacc as bacc
    import concourse.mybir as mybir
    import concourse.tile as tile

    FP32 = mybir.dt.float32
    FP32R = mybir.dt.float32r
    FP16 = mybir.dt.float16
    AF = mybir.ActivationFunctionType

    nc = bacc.Bacc("TRN2", target_bir_lowering=False, debug=False,
                   num_devices=NCORES)
    xT = nc.dram_tensor("xT", [bpc, C, N], FP32, kind="ExternalInput")
    wqkvT = nc.dram_tensor("wqkvT", [C, 3 * C], FP32, kind="ExternalInput")
    wprojT = nc.dram_tensor("wprojT", [C, C], FP32, kind="ExternalInput")
    masks = nc.dram_tensor("masks", [3, 96, 256], FP16, kind="ExternalInput")
    out = nc.dram_tensor("out", [bpc, N, C], FP32, kind="ExternalOutput")

    halves = _half_starts()
    hidx = {s: i for i, s in enumerate(halves)}

    with tile.TileContext(nc) as tc:
        with (
            tc.tile_pool(name="const", bufs=1) as cpool,
            tc.tile_pool(name="xt", bufs=1) as xt_pool,
            tc.tile_pool(name="qk", bufs=1) as qk_pool,
            tc.tile_pool(name="vsb", bufs=1) as v_pool,
            tc.tile_pool(name="vw", bufs=1) as vw_pool,
            tc.tile_pool(name="ot", bufs=1) as ot_pool,
            tc.tile_pool(name="wqs", bufs=3) as wq_pool,
            tc.tile_pool(name="wbig", bufs=1) as wbig_pool,
            tc.tile_pool(name="attn", bufs=4) as attn_pool,
            tc.tile_pool(name="ysb", bufs=2) as y_pool,
            tc.tile_pool(name="psA", bufs=2, space="PSUM") as psA,
            tc.tile_pool(name="psS", bufs=2, space="PSUM") as psS,
            tc.tile_pool(name="psO", bufs=2, space="PSUM") as psO,
        ):
            mask_t = cpool.tile([96, 3 * 256], FP16, tag="masks")
            nc.sync.dma_start(
                out=mask_t[:].rearrange("p (v f) -> p v f", v=3),
                in_=masks.rearrange("v p f -> p v f"))

            for b in range(bpc):
                # ---- load x^T (c on partitions) ----
                xts = []
                for j in range(KO):
                    t_ = xt_pool.tile([128, N], FP32R, tag=f"xt{j}")
                    nc.sync.dma_start(
                        out=t_[:],
                        in_=xT[b, j * 128:(j + 1) * 128, :].bitcast(FP32R))
                    xts.append(t_)

                # ---- q^T, k^T head-pair tiles: (f=128, n=1024) ----
                qks = []
                for p in list(range(heads // 2)) + list(range(8, 8 + heads // 2)):
                    wt = wq_pool.tile([128, KO * 128], FP32R, tag="wq")
                    nc.sync.dma_start(
                        out=wt[:].rearrange("p (ko f) -> p ko f", ko=KO),
                        in_=wqkvT[:, p * 128:(p + 1) * 128]
                        .rearrange("(ko pp) f -> pp ko f", pp=128)
                        .bitcast(FP32R))
                    qkt = qk_pool.tile([128, N], FP16, tag=f"qk{p}")
                    for ch in range(2):
                        ps = psA.tile([128, 512], FP32, tag="psA")
                        for ko in range(KO):
                            nc.tensor.matmul(
                                ps[:],
                                wt[:, ko * 128:(ko + 1) * 128],
                                xts[ko][:, ch * 512:(ch + 1) * 512],
                                start=(ko == 0), stop=(ko == KO - 1))
                        nc.scalar.copy(qkt[:, ch * 512:(ch + 1) * 512], ps[:])
                    qks.append(qkt)
                qk_map = {}
                for i, p in enumerate(list(range(heads // 2)) +
                                      list(range(8, 8 + heads // 2))):
                    qk_map[p] = qks[i]

                # ---- v in natural (n, d) layout ----
                wv = wbig_pool.tile([128, KO * 1024], FP32R, tag="wbig")
                nc.sync.dma_start(
                    out=wv[:].rearrange("p (ko f) -> p ko f", ko=KO),
                    in_=wqkvT[:, 2 * C:3 * C]
                    .rearrange("(ko pp) f -> pp ko f", pp=128)
                    .bitcast(FP32R))
                vts = []
                for t in range(8):
                    vt = v_pool.tile([128, 1024], FP16, tag=f"v{t}")
                    for ch in range(2):
                        ps = psA.tile([128, 512], FP32, tag="psA")
                        for ko in range(KO):
                            nc.tensor.matmul(
                                ps[:],
                                xts[ko][:, t * 128:(t + 1) * 128],
                                wv[:, ko * 1024 + ch * 512:
                                   ko * 1024 + (ch + 1) * 512],
                                start=(ko == 0), stop=(ko == KO - 1))
                        nc.scalar.copy(vt[:, ch * 512:(ch + 1) * 512], ps[:])
                    vts.append(vt)

                # ---- windowed v-halves with ones column per head slot ----
                vws = {}
                for s in halves:
                    vw = vw_pool.tile([96, 16 * 65], FP16, tag=f"vw{hidx[s]}")
                    d3 = vw[:].rearrange("p (h x) -> p h x", x=65)
                    t0, r0 = s // 128, s % 128
                    rows0 = min(128 - r0, 96)
                    nc.sync.dma_start(
                        out=d3[0:rows0, :, 0:64],
                        in_=vts[t0][r0:r0 + rows0, :]
                        .rearrange("p (h x) -> p h x", x=64))
                    if rows0 < 96:
                        nc.sync.dma_start(
                            out=d3[rows0:96, :, 0:64],
                            in_=vts[t0 + 1][0:96 - rows0, :]
                            .rearrange("p (h x) -> p h x", x=64))
                    nc.vector.memset(d3[:, :, 64:65], 1.0)
                    vws[s] = vw

                # ---- attention ----
                ots = []
                for j in range(KO):
                    ot_j = ot_pool.tile([128, N], FP32R, tag=f"ot{j}")
                    ots.append(ot_j)
                for h in range(heads):
                    pq = h // 2
                    off = (h % 2) * 64
                    qt = qk_map[pq]
                    kt = qk_map[8 + pq]
                    for t in range(qtiles):
                        w = _win_start(t)
                        mv = 0 if t == 0 else (2 if t == 7 else 1)
                        ps_s = psS.tile([96, 256], FP32, tag="psS")
                        for half in range(2):
                            s = w + 96 * half
                            nc.tensor.matmul(
                                ps_s[:, half * 128:(half + 1) * 128],
                                kt[off:off + 64, s:s + 96],
                                qt[off:off + 64, t * 128:(t + 1) * 128],
                                start=True, stop=True)
                        e = attn_pool.tile([96, 256], FP16, tag="e")
                        nc.scalar.activation(e[:], ps_s[:], AF.Exp, scale=0.125)
                        pt = attn_pool.tile([96, 256], FP16, tag="p")
                        nc.vector.tensor_mul(
                            pt[:], e[:], mask_t[:, mv * 256:(mv + 1) * 256])
                        ps_o = psO.tile([65, 128], FP32, tag="psO")
                        for half in range(2):
                            s = w + 96 * half
                            v3 = vws[s][:].rearrange("p (h x) -> p h x", x=65)
                            nc.tensor.matmul(
                                ps_o[:],
                                v3[:, h, :],
                                pt[:, half * 128:(half + 1) * 128],
                                start=(half == 0), stop=(half == 1))
                        linv = attn_pool.tile([1, 128], FP32, tag="linv")
                        nc.vector.reciprocal(linv[:], ps_o[64:65, :])
                        lb = attn_pool.tile([64, 128], FP32, tag="lb")
                        nc.gpsimd.partition_broadcast(lb[:], linv[:])
                        nc.vector.tensor_mul(
                            ots[pq][off:off + 64, t * 128:(t + 1) * 128],
                            ps_o[0:64, :], lb[:])

                # ---- output projection ----
                wp = wbig_pool.tile([128, KO * 1024], FP32R, tag="wbig")
                nc.sync.dma_start(
                    out=wp[:].rearrange("p (ko f) -> p ko f", ko=KO),
                    in_=wprojT.rearrange("(ko pp) f -> pp ko f", pp=128)
                    .bitcast(FP32R))
                for t in range(8):
                    y = y_pool.tile([128, 1024], FP32, tag="y")
                    for ch in range(2):
                        ps = psA.tile([128, 512], FP32, tag="psA")
                        for ko in range(KO):
                            nc.tensor.matmul(
                                ps[:],
                                ots[ko][:, t * 128:(t + 1) * 128],
                                wp[:, ko * 1024 + ch * 512:
                                   ko * 1024 + (ch + 1) * 512],
                                start=(ko == 0), stop=(ko == KO - 1))
                        nc.vector.tensor_copy(y[:, ch * 512:(ch + 1) * 512],
                                              ps[:])
                    nc.sync.dma_start(out=out[b, t * 128:(t + 1) * 128, :],
                                      in_=y[:])

    nc.compile()
    return nc


def _get_nc():
    if "nc" not in _cache:
        _cache["nc"] = build_nc()
    return _cache["nc"]


def _make_in_maps(x, w_qkv, w_proj):
    import ml_dtypes
    xT = np.ascontiguousarray(np.asarray(x, dtype=np.float32).transpose(0, 2, 1))
    wqkvT = np.ascontiguousarray(np.asarray(w_qkv, dtype=np.float32).T)
    wprojT = np.ascontiguousarray(np.asarray(w_proj, dtype=np.float32).T)
    del ml_dtypes
    masks = _build_masks(np.float16)
    in_maps = []
    for c in range(NCORES):
        in_maps.append({
            "xT": xT[c * BPC:(c + 1) * BPC],
            "wqkvT": wqkvT,
            "wprojT": wprojT,
            "masks": masks,
        })
    return in_maps


def kernel(x, w_qkv, w_proj, b_proj):
    from concourse.bass_utils import run_bass_kernel_spmd

    nc = _get_nc()
    in_maps = _make_in_maps(x, w_qkv, w_proj)
    r = run_bass_kernel_spmd(nc, in_maps, core_ids=list(range(NCORES)))
    y = np.concatenate([r.results[c]["out"] for c in range(NCORES)], axis=0)
    y = y + np.asarray(b_proj, dtype=np.float32)[None, None, :]
    return y.astype(np.float32)
